# revision 4
# baseline (speedup 1.0000x reference)
"""Trainium2 Bass kernel for nn_Attention (LayerNorm + L2-normalized-QK attention
with null-kv slot + output projection), SPMD across 8 NeuronCores.

Sharding: core c = (batch b = c//2, query-half hi = c%2). Each core computes the
full kv (2048 tokens) of its batch and attention outputs for its 1024-query
half. Softmax over kv is permutation invariant, so for hi=1 we feed x with the
two sequence halves swapped — every core then runs the identical SPMD program
with its queries in rows 0:1024. The final output is a pure concatenation of
the per-core results (no collectives, no host arithmetic on outputs).

v2 pipeline layout (vs v1): LayerNorm is pipelined per 512-token group into
the projections so the PE never idles at the head; gamma is folded into the
projection weights on the host and weights are shipped pre-tiled in bf16 (no
on-device weight casts); the l2norm rsqrt runs per tile (Ln+Exp live in one
ACT table set); PSUM->SBUF projection copies are fused into the normalization
multiplies; v-scaling moves to the scalar engine; the softmax denominator
broadcast runs on the (otherwise idle) gpsimd engine instead of DRAM
round-trips; and the qb0 output projection is interleaved into qb1 attention.
"""

import numpy as np

B = 4
N = 2048
DIM = 1024
HEADS = 16
DH = 64
INNER = HEADS * DH
NQ = 1024  # queries per core
SCALE = 8.0
LN_EPS = 1e-5

_CACHE = {}


def _build_program():
    from contextlib import ExitStack

    import concourse.bacc as bacc
    import concourse.tile as tile
    from concourse import mybir

    f32 = mybir.dt.float32
    bf16 = mybir.dt.bfloat16
    AF = mybir.ActivationFunctionType
    OP = mybir.AluOpType
    AX = mybir.AxisListType

    NT = N // 128          # 16 token tiles
    NTQ = NQ // 128        # 8 query token tiles
    NCD = DIM // 128       # 8 dim chunks
    HP = HEADS // 2        # 8 head pairs
    NG = 4                 # 512-token LN groups

    nc = bacc.Bacc("TRN2", target_bir_lowering=False, debug=False)

    x = nc.declare_dram_parameter("x", [N, DIM], f32, isOutput=False)
    # weights arrive pre-tiled [128, NCD, INNER] bf16, gamma pre-folded
    Wq = nc.declare_dram_parameter("Wq", [128, NCD, INNER], bf16, isOutput=False)
    Wk = nc.declare_dram_parameter("Wk", [128, NCD, INNER], bf16, isOutput=False)
    Wv = nc.declare_dram_parameter("Wv", [128, NCD, INNER], bf16, isOutput=False)
    Wo = nc.declare_dram_parameter("Wo", [128, NCD, DIM], bf16, isOutput=False)
    nk = nc.declare_dram_parameter("nk", [HEADS, DH], f32, isOutput=False)
    nv = nc.declare_dram_parameter("nv", [HEADS, DH], f32, isOutput=False)
    qs = nc.declare_dram_parameter("qs", [DH], f32, isOutput=False)
    ks = nc.declare_dram_parameter("ks", [DH], f32, isOutput=False)
    out = nc.declare_dram_parameter("out", [NQ, DIM], f32, isOutput=True)

    # internal DRAM for transpose round-trips
    xn_d = nc.dram_tensor("xn_d", [N, DIM], bf16)
    kn_d = nc.dram_tensor("kn_d", [N, INNER], bf16)
    qn_d = nc.dram_tensor("qn_d", [NQ, INNER], bf16)
    nkn_d = nc.dram_tensor("nkn_d", [HEADS, DH], bf16)
    nvb_d = nc.dram_tensor("nvb_d", [HEADS, DH + 1], bf16)

    with tile.TileContext(nc) as tc, ExitStack() as ctx:
        singles = ctx.enter_context(tc.tile_pool(name="singles", bufs=1))
        big = ctx.enter_context(tc.tile_pool(name="big", bufs=1))

        # ---------------- persistent SBUF tensors ----------------
        xnT = big.tile([128, NCD, N], bf16, tag="xnT")       # (x-mu)^T [dim, tok]
        kT = big.tile([128, HP, N], bf16, tag="kT")          # k^T   [2*64, kv] per pair
        qT = big.tile([128, HP, NQ], bf16, tag="qT")         # q^T
        vsb = big.tile([128, NT, HEADS, DH + 1], bf16, tag="v")   # V'=[V|1]
        AT = big.tile([128, NCD, NQ], bf16, tag="AT")        # A^T (attn out)
        wo_sb = big.tile([128, NCD, DIM], bf16, tag="wo")    # Wo resident

        # ---------------- constants ----------------
        eps_t = singles.tile([128, 1], f32)
        nc.vector.memset(eps_t, LN_EPS)
        eps30 = singles.tile([128, 1], f32)
        nc.vector.memset(eps30, 1e-30)

        qs_b = singles.tile([128, DH], f32)
        nc.gpsimd.dma_start(out=qs_b, in_=qs.ap().partition_broadcast(128))
        ks_b = singles.tile([128, DH], f32)
        nc.gpsimd.dma_start(out=ks_b, in_=ks.ap().partition_broadcast(128))
        c64 = singles.tile([128, DH], f32)
        nc.vector.tensor_tensor(out=c64, in0=qs_b, in1=ks_b, op=OP.mult)
        c8 = singles.tile([128, 8, DH], f32)   # qs*ks tiled for 8 heads (one col half)
        for g in range(8):
            nc.vector.tensor_copy(out=c8[:, g, :], in_=c64)

        nc.vector.memset(vsb[:, :, :, DH : DH + 1], 1.0)  # ones column of V'

        # Wo resident load (2 DMAs on the gpsimd queue, early)
        for half in range(2):
            nc.gpsimd.dma_start(
                out=wo_sb[:, :, half * 512 : (half + 1) * 512],
                in_=Wo[:, :, half * 512 : (half + 1) * 512],
            )

        # null-kv prep: nkn = l2norm(nk)*qs*ks (bf16), then transpose via DRAM
        nk_t = singles.tile([HEADS, DH], f32)
        nc.sync.dma_start(out=nk_t, in_=nk[:, :])
        nksq = singles.tile([HEADS, DH], f32)
        nc.vector.tensor_tensor(out=nksq, in0=nk_t, in1=nk_t, op=OP.mult)
        nks = singles.tile([HEADS, 1], f32)
        nc.vector.tensor_reduce(out=nks, in_=nksq, axis=AX.X, op=OP.add)
        nc.scalar.activation(out=nks, in_=nks, func=AF.Ln, bias=eps30[0:HEADS, :])
        nc.scalar.activation(out=nks, in_=nks, func=AF.Exp, scale=-0.5)
        nc.vector.tensor_scalar_min(out=nks, in0=nks, scalar1=1e12)
        nkn = singles.tile([HEADS, DH], f32)
        nc.vector.tensor_scalar_mul(out=nkn, in0=nk_t, scalar1=nks)
        nknb = singles.tile([HEADS, DH], bf16)
        nc.vector.tensor_tensor(out=nknb, in0=nkn, in1=c64[0:HEADS, :], op=OP.mult)
        nc.sync.dma_start(out=nkn_d[:, :], in_=nknb)
        nknT = singles.tile([DH, HEADS], bf16)
        nc.sync.dma_start(out=nknT, in_=nkn_d.ap().rearrange("h d -> d h"))
        # block-diagonal [128, 16]: col 2p rows 0:64 = head 2p, col 2p+1 rows 64:128 = head 2p+1
        nkn_bd = singles.tile([128, HEADS], bf16)
        nc.vector.memset(nkn_bd, 0.0)
        nc.sync.dma_start(out=nkn_bd[0:DH, 0:HEADS:2], in_=nknT[:, 0:HEADS:2])
        nc.sync.dma_start(out=nkn_bd[DH:128, 1:HEADS:2], in_=nknT[:, 1:HEADS:2])

        # null-v: nv_bd2 [2, HEADS, DH+1]; row parity selects head parity
        nv_t = singles.tile([HEADS, DH], f32)
        nc.sync.dma_start(out=nv_t, in_=nv[:, :])
        nvb = singles.tile([HEADS, DH + 1], bf16)
        nc.vector.tensor_copy(out=nvb[:, 0:DH], in_=nv_t)
        nc.vector.memset(nvb[:, DH : DH + 1], 1.0)
        nv_bd2 = singles.tile([2, HEADS, DH + 1], bf16)
        nc.vector.memset(nv_bd2, 0.0)
        nc.sync.dma_start(out=nvb_d[:, :], in_=nvb)
        nc.sync.dma_start(
            out=nv_bd2[0:1, 0:HEADS:2, :],
            in_=nvb_d.ap()[0:HEADS:2, :].partition_broadcast(1),
        )
        nc.sync.dma_start(
            out=nv_bd2[1:2, 1:HEADS:2, :],
            in_=nvb_d.ap()[1:HEADS:2, :].partition_broadcast(1),
        )

        # ---------------- LayerNorm, pipelined per 512-token group ----------
        # beta==0 fast path: xnu = x - mu. gamma is folded into the weights;
        # the per-token rstd commutes through the projections (q/k l2norms
        # cancel it; the v-proj re-applies it as a per-partition scale).
        mv_all = singles.tile([128, NT, 2], f32)
        rst_all = singles.tile([128, NT], f32)
        with (
            tc.tile_pool(name="px", bufs=3) as px,
            tc.tile_pool(name="pst", bufs=4) as pst,
            tc.tile_pool(name="pxn", bufs=3) as pxn,
        ):
            for tt in range(NT):
                r0 = tt * 128
                xt = px.tile([128, DIM], f32)
                nc.sync.dma_start(out=xt, in_=x[r0 : r0 + 128, :])
                stats = pst.tile([128, 2, 6], f32, tag="stats")
                nc.vector.bn_stats(out=stats[:, 0, :], in_=xt[:, 0:512])
                nc.vector.bn_stats(out=stats[:, 1, :], in_=xt[:, 512:1024])
                nc.vector.bn_aggr(out=mv_all[:, tt, :], in_=stats)
                xnt = pxn.tile([128, DIM], bf16)
                nc.vector.tensor_scalar_sub(
                    out=xnt, in0=xt, scalar1=mv_all[:, tt, 0:1]
                )
                nc.sync.dma_start(out=xn_d[r0 : r0 + 128, :], in_=xnt)
                if tt % 4 == 3:
                    g = tt // 4
                    g0 = (tt - 3) * 128
                    for c in range(NCD):
                        nc.sync.dma_start(
                            out=xnT[:, c, g0 : g0 + 512],
                            in_=xn_d[g0 : g0 + 512, c * 128 : (c + 1) * 128],
                            transpose=True,
                        )
                    # rstd for this group's 4 tiles (only v-proj consumes it)
                    nc.scalar.activation(
                        out=rst_all[:, g * 4 : g * 4 + 4],
                        in_=mv_all[:, g * 4 : g * 4 + 4, 1],
                        func=AF.Ln, bias=eps_t,
                    )
                    nc.scalar.activation(
                        out=rst_all[:, g * 4 : g * 4 + 4],
                        in_=rst_all[:, g * 4 : g * 4 + 4],
                        func=AF.Exp, scale=-0.5,
                    )

        # ---------------- projections ----------------
        with (
            tc.tile_pool(name="pw", bufs=2) as pw,
            tc.tile_pool(name="ppj", bufs=4, space="PSUM") as ppj,
            tc.tile_pool(name="pnrm", bufs=4) as pnrm,
        ):
            def load_w_half(W, half):
                wt = pw.tile([128, NCD, 512], bf16, tag="W")
                nc.gpsimd.dma_start(
                    out=wt, in_=W[:, :, half * 512 : (half + 1) * 512]
                )
                return wt

            def proj_norm_tiles(W, half, n_tiles, dst, with_c8):
                """q/k projection + per-tile l2norm scale -> dst DRAM (bf16)."""
                wt = load_w_half(W, half)
                for tt in range(n_tiles):
                    r0 = tt * 128
                    kp = ppj.tile([128, 512], f32, tag="pj")
                    for c in range(NCD):
                        nc.tensor.matmul(
                            kp, lhsT=xnT[:, c, r0 : r0 + 128], rhs=wt[:, c, :],
                            start=(c == 0), stop=(c == NCD - 1),
                        )
                    sq = pnrm.tile([128, 512], f32, tag="sq")
                    nc.scalar.activation(out=sq, in_=kp, func=AF.Square)
                    rs = pnrm.tile([128, 8], f32, tag="rs")
                    nc.vector.tensor_reduce(
                        out=rs, in_=sq.rearrange("p (g d) -> p g d", g=8),
                        axis=AX.X, op=OP.add,
                    )
                    nc.scalar.activation(out=rs, in_=rs, func=AF.Ln, bias=eps30)
                    nc.scalar.activation(out=rs, in_=rs, func=AF.Exp, scale=-0.5)
                    nc.vector.tensor_scalar_min(out=rs, in0=rs, scalar1=1e12)
                    if with_c8:
                        rex = pnrm.tile([128, 8, DH], f32, tag="rex")
                        nc.vector.tensor_tensor(
                            out=rex,
                            in0=rs.broadcast_to([128, 8, DH]),
                            in1=c8, op=OP.mult,
                        )
                        knf = pnrm.tile([128, 512], bf16, tag="knf")
                        nc.vector.tensor_tensor(
                            out=knf.rearrange("p (g d) -> p g d", g=8),
                            in0=kp.rearrange("p (g d) -> p g d", g=8),
                            in1=rex, op=OP.mult,
                        )
                    else:
                        knf = pnrm.tile([128, 512], bf16, tag="knf")
                        nc.vector.tensor_tensor(
                            out=knf.rearrange("p (g d) -> p g d", g=8),
                            in0=kp.rearrange("p (g d) -> p g d", g=8),
                            in1=rs.broadcast_to([128, 8, DH]),
                            op=OP.mult,
                        )
                    nc.sync.dma_start(
                        out=dst[r0 : r0 + 128, half * 512 : (half + 1) * 512],
                        in_=knf,
                    )

            def proj_v_tiles(half):
                wt = load_w_half(Wv, half)
                for tt in range(NT):
                    r0 = tt * 128
                    vp = ppj.tile([128, 512], f32, tag="pj")
                    for c in range(NCD):
                        nc.tensor.matmul(
                            vp, lhsT=xnT[:, c, r0 : r0 + 128], rhs=wt[:, c, :],
                            start=(c == 0), stop=(c == NCD - 1),
                        )
                    # v * rstd (per-token) on the scalar engine
                    nc.scalar.activation(
                        out=vsb[:, tt, half * 8 : (half + 1) * 8, 0:DH],
                        in_=vp.rearrange("p (g d) -> p g d", g=8),
                        func=AF.Copy, scale=rst_all[:, tt : tt + 1],
                    )

            for half in range(2):
                proj_norm_tiles(Wk, half, NT, kn_d, with_c8=False)
                proj_norm_tiles(Wq, half, NTQ, qn_d, with_c8=True)
                proj_v_tiles(half)
                # transposes for this half's head pairs
                for p in range(half * 4, half * 4 + 4):
                    nc.sync.dma_start(
                        out=kT[:, p, :], in_=kn_d[:, p * 128 : (p + 1) * 128],
                        transpose=True,
                    )
                    nc.sync.dma_start(
                        out=qT[:, p, :], in_=qn_d[:, p * 128 : (p + 1) * 128],
                        transpose=True,
                    )

        # ---------------- attention + interleaved output projection --------
        QB = NQ // 512  # 2 query blocks of 512
        with (
            tc.tile_pool(name="pstt", bufs=2, space="PSUM") as pstt,
            tc.tile_pool(name="pot", bufs=3, space="PSUM") as pot,
            tc.tile_pool(name="ppj2", bufs=1, space="PSUM") as ppj2,
            tc.tile_pool(name="ppt", bufs=3) as ppt,
            tc.tile_pool(name="pptn", bufs=2) as pptn,
            tc.tile_pool(name="prec", bufs=4) as prec,
            tc.tile_pool(name="pbsc", bufs=4) as pbsc,
            tc.tile_pool(name="pob", bufs=2) as pob,
        ):
            def outproj_block(i):
                """Output projection for the i-th [128-token x 512-col] block
                of qb0 (i in 0..7): token tile i//2, column half i%2."""
                tt, half = divmod(i, 2)
                r0 = tt * 128
                op_ = ppj2.tile([128, 512], f32)
                for c in range(NCD):
                    nc.tensor.matmul(
                        op_, lhsT=AT[:, c, r0 : r0 + 128],
                        rhs=wo_sb[:, c, half * 512 : (half + 1) * 512],
                        start=(c == 0), stop=(c == NCD - 1),
                    )
                ob = pob.tile([128, 512], f32)
                nc.vector.tensor_copy(out=ob, in_=op_)
                nc.sync.dma_start(
                    out=out[r0 : r0 + 128, half * 512 : (half + 1) * 512], in_=ob
                )

            for qb in range(QB):
                q0 = qb * 512
                for hp in range(HP):
                    hA, hB = 2 * hp, 2 * hp + 1
                    # null scores for both heads: [2, 512] (in an st slot)
                    st_n = pstt.tile([128, 2, 512], f32, tag="st")
                    null_ps = st_n[0:2, 0, :]
                    nc.tensor.matmul(
                        null_ps, lhsT=nkn_bd[:, hA : hA + 2],
                        rhs=qT[:, hp, q0 : q0 + 512], start=True, stop=True,
                    )
                    pTn = pptn.tile([2, 512], bf16)
                    nc.scalar.activation(out=pTn, in_=null_ps, func=AF.Exp, scale=SCALE)

                    otA = pot.tile([DH + 1, 512], f32, tag="ot")
                    otB = pot.tile([DH + 1, 512], f32, tag="ot")

                    for c in range(16):
                        st = pstt.tile([128, 2, 512], f32, tag="st")
                        for si, rh in ((0, 0), (1, 1)):
                            nc.tensor.matmul(
                                st[:, si, :],
                                lhsT=kT[rh * DH : (rh + 1) * DH, hp, c * 128 : (c + 1) * 128],
                                rhs=qT[rh * DH : (rh + 1) * DH, hp, q0 : q0 + 512],
                                start=True, stop=True,
                                tile_position=(rh * DH, 0),
                            )
                        pt = ppt.tile([128, 2, 512], bf16)
                        nc.scalar.activation(out=pt, in_=st, func=AF.Exp, scale=SCALE)
                        for ot, h in ((otA, hA), (otB, hB)):
                            nc.tensor.matmul(
                                ot, lhsT=vsb[:, c, h, :], rhs=pt[:, h % 2, :],
                                start=(c == 0), stop=False,
                            )
                    # null PV (finishes accumulation)
                    nc.tensor.matmul(
                        otA, lhsT=nv_bd2[:, hA, :], rhs=pTn, start=False, stop=True
                    )
                    nc.tensor.matmul(
                        otB, lhsT=nv_bd2[:, hB, :], rhs=pTn, start=False, stop=True
                    )
                    # reciprocal of denominators (row DH), broadcast on gpsimd,
                    # apply, and write A^T
                    rcpA = prec.tile([1, 512], f32, tag="rcpA")
                    rcpB = prec.tile([1, 512], f32, tag="rcpB")
                    nc.vector.reciprocal(rcpA, otA[DH : DH + 1, :])
                    nc.vector.reciprocal(rcpB, otB[DH : DH + 1, :])
                    for rcp1, (h, ot) in ((rcpA, (hA, otA)), (rcpB, (hB, otB))):
                        rcs = pbsc.tile([DH, 512], f32, tag="bcs")
                        nc.gpsimd.partition_broadcast(rcs, rcp1)
                        po = (h % 2) * DH
                        nc.vector.tensor_tensor(
                            out=AT[po : po + DH, h // 2, q0 : q0 + 512],
                            in0=ot[0:DH, :], in1=rcs, op=OP.mult,
                        )
                    # interleave qb0's output projection into qb1's attention
                    if qb == 1:
                        outproj_block(hp)
            # tail: qb1's output projection
            for i in range(8):
                tt, half = divmod(i, 2)
                r0 = 512 + tt * 128
                op_ = ppj2.tile([128, 512], f32)
                for c in range(NCD):
                    nc.tensor.matmul(
                        op_, lhsT=AT[:, c, r0 : r0 + 128],
                        rhs=wo_sb[:, c, half * 512 : (half + 1) * 512],
                        start=(c == 0), stop=(c == NCD - 1),
                    )
                ob = pob.tile([128, 512], f32)
                nc.vector.tensor_copy(out=ob, in_=op_)
                nc.sync.dma_start(
                    out=out[r0 : r0 + 128, half * 512 : (half + 1) * 512], in_=ob
                )

    nc.compile()
    return nc


def _get_program(beta_zero: bool = True):
    key = "nc_v2"
    if key not in _CACHE:
        _CACHE[key] = _build_program()
    return _CACHE[key]


def _prep_weights(Wq, Wkv, Wo, gamma, beta):
    """Host-side: fold gamma into the projection weights, pre-tile to
    [128, NCD, cols] (chunk c holds dim rows c*128:(c+1)*128), cast bf16."""
    import ml_dtypes

    NCD = DIM // 128
    Wk = Wkv[:, :INNER]
    Wv = Wkv[:, INNER:]
    g = gamma.astype(np.float64)[:, None]

    def tile_w(W, fold_gamma=True):
        Wf = W.astype(np.float64) * g if fold_gamma else W.astype(np.float64)
        t = Wf.reshape(NCD, 128, Wf.shape[1]).transpose(1, 0, 2)
        return np.ascontiguousarray(t.astype(ml_dtypes.bfloat16))

    return (
        tile_w(Wq), tile_w(Wk), tile_w(Wv), tile_w(Wo, fold_gamma=False),
    )


def kernel(**inputs) -> np.ndarray:
    from concourse.bass_utils import run_bass_kernel_spmd

    x = np.asarray(inputs["x"], dtype=np.float32)
    gamma = np.asarray(inputs["gamma"], dtype=np.float32)
    beta = np.asarray(inputs["beta"], dtype=np.float32)
    null_kv = np.asarray(inputs["null_kv"], dtype=np.float32)
    Wq = np.asarray(inputs["Wq"], dtype=np.float32)
    Wkv = np.asarray(inputs["Wkv"], dtype=np.float32)
    q_scale = np.asarray(inputs["q_scale"], dtype=np.float32)
    k_scale = np.asarray(inputs["k_scale"], dtype=np.float32)
    Wo = np.asarray(inputs["Wo"], dtype=np.float32)

    if np.any(beta):
        # General-path fallback: beta shifts xn, which the fast path folds
        # away. Absorb beta into x directly: xn = ((x-mu)/std)*gamma + beta
        # is NOT linear in x, so instead fall back to adding beta/gamma
        # pre-projection is invalid; handle by explicit correction below.
        raise NotImplementedError("beta != 0 not supported by this kernel")

    nc = _get_program()
    Wq_t, Wk_t, Wv_t, Wo_t = _prep_weights(Wq, Wkv, Wo, gamma, beta)
    nk = np.ascontiguousarray(null_kv[0, :, 0, :])
    nv = np.ascontiguousarray(null_kv[1, :, 0, :])

    in_maps = []
    for b in range(B):
        for hi in range(2):
            xb = x[b]
            if hi == 1:
                xb = np.concatenate([xb[NQ:], xb[:NQ]], axis=0)
            in_maps.append(
                {
                    "x": np.ascontiguousarray(xb),
                    "Wq": Wq_t,
                    "Wk": Wk_t,
                    "Wv": Wv_t,
                    "Wo": Wo_t,
                    "nk": nk,
                    "nv": nv,
                    "qs": q_scale,
                    "ks": k_scale,
                }
            )

    res = run_bass_kernel_spmd(nc, in_maps, list(range(8)))

    full = np.empty((B, N, DIM), dtype=np.float32)
    for c in range(8):
        b, hi = divmod(c, 2)
        full[b, hi * NQ : (hi + 1) * NQ] = res.results[c]["out"]
    return full


# revision 13
# speedup vs baseline: 1.4474x; 1.4474x over previous
"""Trainium2 Bass kernel for nn_Attention (LayerNorm + L2-normalized-QK attention
with null-kv slot + output projection), SPMD across 8 NeuronCores.

Sharding: core c = (batch b = c//2, query-half hi = c%2). Each core computes the
full kv (2048 tokens) of its batch and attention outputs for its 1024-query
half. Softmax over kv is permutation invariant, so for hi=1 we feed x with the
two sequence halves swapped — every core then runs the identical SPMD program
with its queries in rows 0:1024. The final output is a pure concatenation of
the per-core results (no collectives, no host arithmetic on outputs).

v2 pipeline layout (vs v1): LayerNorm is pipelined per 512-token group into
the projections so the PE never idles at the head; gamma is folded into the
projection weights on the host and weights are shipped pre-tiled in bf16 (no
on-device weight casts); the l2norm rsqrt runs per tile (Ln+Exp live in one
ACT table set); PSUM->SBUF projection copies are fused into the normalization
multiplies; v-scaling moves to the scalar engine; the softmax denominator
broadcast runs on the (otherwise idle) gpsimd engine instead of DRAM
round-trips; and the qb0 output projection is interleaved into qb1 attention.
"""

import numpy as np

B = 4
N = 2048
DIM = 1024
HEADS = 16
DH = 64
INNER = HEADS * DH
NQ = 1024  # queries per core
SCALE = 8.0
LN_EPS = 1e-5

_CACHE = {}


def _build_program():
    from contextlib import ExitStack

    import concourse.bacc as bacc
    import concourse.tile as tile
    from concourse import mybir

    f32 = mybir.dt.float32
    bf16 = mybir.dt.bfloat16
    AF = mybir.ActivationFunctionType
    OP = mybir.AluOpType
    AX = mybir.AxisListType

    NT = N // 128          # 16 token tiles
    NTQ = NQ // 128        # 8 query token tiles
    NCD = DIM // 128       # 8 dim chunks
    HP = HEADS // 2        # 8 head pairs
    NG = 4                 # 512-token LN groups

    nc = bacc.Bacc("TRN2", target_bir_lowering=False, debug=False)

    x = nc.declare_dram_parameter("x", [N, DIM], f32, isOutput=False)
    # weights arrive pre-tiled [128, NCD, INNER] bf16, gamma pre-folded
    Wq = nc.declare_dram_parameter("Wq", [128, NCD, INNER], bf16, isOutput=False)
    Wk = nc.declare_dram_parameter("Wk", [128, NCD, INNER], bf16, isOutput=False)
    Wv = nc.declare_dram_parameter("Wv", [128, NCD, INNER], bf16, isOutput=False)
    Wo = nc.declare_dram_parameter("Wo", [128, NCD, DIM], bf16, isOutput=False)
    nk = nc.declare_dram_parameter("nk", [HEADS, DH], f32, isOutput=False)
    nv = nc.declare_dram_parameter("nv", [HEADS, DH], f32, isOutput=False)
    qs = nc.declare_dram_parameter("qs", [DH], f32, isOutput=False)
    ks = nc.declare_dram_parameter("ks", [DH], f32, isOutput=False)
    out = nc.declare_dram_parameter("out", [NQ, DIM], f32, isOutput=True)

    # internal DRAM for transpose round-trips
    xn_d = nc.dram_tensor("xn_d", [N, DIM], bf16)
    kn_d = nc.dram_tensor("kn_d", [N, INNER], bf16)
    qn_d = nc.dram_tensor("qn_d", [NQ, INNER], bf16)
    nkn_d = nc.dram_tensor("nkn_d", [HEADS, DH], bf16)
    nvb_d = nc.dram_tensor("nvb_d", [HEADS, DH + 1], bf16)

    with tile.TileContext(nc) as tc, ExitStack() as ctx:
        singles = ctx.enter_context(tc.tile_pool(name="singles", bufs=1))
        big = ctx.enter_context(tc.tile_pool(name="big", bufs=1))

        # ---------------- persistent SBUF tensors ----------------
        xnT = big.tile([128, NCD, N], bf16, tag="xnT")       # (x-mu)^T [dim, tok]
        kT = big.tile([128, HP, N], bf16, tag="kT")          # k^T   [2*64, kv] per pair
        qT = big.tile([128, HP, NQ], bf16, tag="qT")         # q^T
        vsb = big.tile([128, NT, HEADS, DH + 1], bf16, tag="v")   # V'=[V|1]
        AT = big.tile([128, NCD, NQ], bf16, tag="AT")        # A^T (attn out)
        wo_sb = big.tile([128, NCD, DIM], bf16, tag="wo")    # Wo resident

        # ---------------- constants ----------------
        eps_t = singles.tile([128, 1], f32)
        nc.vector.memset(eps_t, LN_EPS)
        eps30 = singles.tile([128, 1], f32)
        nc.vector.memset(eps30, 1e-30)

        qs_b = singles.tile([128, DH], f32)
        nc.gpsimd.dma_start(out=qs_b, in_=qs.ap().partition_broadcast(128))
        ks_b = singles.tile([128, DH], f32)
        nc.gpsimd.dma_start(out=ks_b, in_=ks.ap().partition_broadcast(128))
        c64 = singles.tile([128, DH], f32)
        nc.vector.tensor_tensor(out=c64, in0=qs_b, in1=ks_b, op=OP.mult)
        c8 = singles.tile([128, 8, DH], f32)   # qs*ks tiled for 8 heads (one col half)
        for g in range(8):
            nc.vector.tensor_copy(out=c8[:, g, :], in_=c64)

        nc.vector.memset(vsb[:, :, :, DH : DH + 1], 1.0)  # ones column of V'

        # Wo resident load (2 DMAs on the gpsimd queue, early)
        for half in range(2):
            nc.gpsimd.dma_start(
                out=wo_sb[:, :, half * 512 : (half + 1) * 512],
                in_=Wo[:, :, half * 512 : (half + 1) * 512],
            )

        # null-kv prep: nkn = l2norm(nk)*qs*ks (bf16), then transpose via DRAM
        nk_t = singles.tile([HEADS, DH], f32)
        nc.sync.dma_start(out=nk_t, in_=nk[:, :])
        nksq = singles.tile([HEADS, DH], f32)
        nc.vector.tensor_tensor(out=nksq, in0=nk_t, in1=nk_t, op=OP.mult)
        nks = singles.tile([HEADS, 1], f32)
        nc.vector.tensor_reduce(out=nks, in_=nksq, axis=AX.X, op=OP.add)
        nkr = singles.tile([HEADS, 1], f32)
        nc.scalar.activation(out=nkr, in_=nks, func=AF.Sqrt, bias=eps30[0:HEADS, :])
        nc.vector.reciprocal(nks, nkr)
        nc.vector.tensor_scalar_min(out=nks, in0=nks, scalar1=1e12)
        nkn = singles.tile([HEADS, DH], f32)
        nc.vector.tensor_scalar_mul(out=nkn, in0=nk_t, scalar1=nks)
        nknb = singles.tile([HEADS, DH], bf16)
        nc.vector.tensor_tensor(out=nknb, in0=nkn, in1=c64[0:HEADS, :], op=OP.mult)
        nc.sync.dma_start(out=nkn_d[:, :], in_=nknb)
        nknT = singles.tile([DH, HEADS], bf16)
        nc.sync.dma_start(out=nknT, in_=nkn_d.ap().rearrange("h d -> d h"))
        # block-diagonal [128, 16]: col 2p rows 0:64 = head 2p, col 2p+1 rows 64:128 = head 2p+1
        nkn_bd = singles.tile([128, HEADS], bf16)
        nc.vector.memset(nkn_bd, 0.0)
        nc.sync.dma_start(out=nkn_bd[0:DH, 0:HEADS:2], in_=nknT[:, 0:HEADS:2])
        nc.sync.dma_start(out=nkn_bd[DH:128, 1:HEADS:2], in_=nknT[:, 1:HEADS:2])

        # null-v: nv_bd2 [2, HEADS, DH+1]; row parity selects head parity
        nv_t = singles.tile([HEADS, DH], f32)
        nc.sync.dma_start(out=nv_t, in_=nv[:, :])
        nvb = singles.tile([HEADS, DH + 1], bf16)
        nc.vector.tensor_copy(out=nvb[:, 0:DH], in_=nv_t)
        nc.vector.memset(nvb[:, DH : DH + 1], 1.0)
        nv_bd2 = singles.tile([2, HEADS, DH + 1], bf16)
        nc.vector.memset(nv_bd2, 0.0)
        nc.sync.dma_start(out=nvb_d[:, :], in_=nvb)
        nc.sync.dma_start(
            out=nv_bd2[0:1, 0:HEADS:2, :],
            in_=nvb_d.ap()[0:HEADS:2, :].partition_broadcast(1),
        )
        nc.sync.dma_start(
            out=nv_bd2[1:2, 1:HEADS:2, :],
            in_=nvb_d.ap()[1:HEADS:2, :].partition_broadcast(1),
        )

        # ---------------- LayerNorm, pipelined per 512-token group ----------
        # beta==0 fast path: xnu = x - mu. gamma is folded into the weights;
        # the per-token rstd commutes through the projections (q/k l2norms
        # cancel it; the v-proj re-applies it as a per-partition scale).
        mv_all = singles.tile([128, NT, 2], f32)
        rst_all = singles.tile([128, NT], f32)
        with (
            tc.tile_pool(name="px", bufs=3) as px,
            tc.tile_pool(name="pst", bufs=4) as pst,
            tc.tile_pool(name="pxn", bufs=3) as pxn,
        ):
            for tt in range(NT):
                r0 = tt * 128
                xt = px.tile([128, DIM], f32)
                nc.sync.dma_start(out=xt, in_=x[r0 : r0 + 128, :])
                stats = pst.tile([128, 2, 6], f32, tag="stats")
                nc.vector.bn_stats(out=stats[:, 0, :], in_=xt[:, 0:512])
                nc.vector.bn_stats(out=stats[:, 1, :], in_=xt[:, 512:1024])
                nc.vector.bn_aggr(out=mv_all[:, tt, :], in_=stats)
                xnt = pxn.tile([128, DIM], bf16)
                nc.vector.tensor_scalar_sub(
                    out=xnt, in0=xt, scalar1=mv_all[:, tt, 0:1]
                )
                nc.scalar.dma_start(out=xn_d[r0 : r0 + 128, :], in_=xnt)
                if tt % 4 == 3:
                    g = tt // 4
                    g0 = (tt - 3) * 128
                    for c in range(NCD):
                        nc.scalar.dma_start(
                            out=xnT[:, c, g0 : g0 + 512],
                            in_=xn_d[g0 : g0 + 512, c * 128 : (c + 1) * 128],
                            transpose=True,
                        )
                    # rstd for this group's 4 tiles (only v-proj consumes it)
                    sd4 = pst.tile([128, 4], f32, tag="sd4")
                    nc.scalar.activation(
                        out=sd4,
                        in_=mv_all[:, g * 4 : g * 4 + 4, 1],
                        func=AF.Sqrt, bias=eps_t,
                    )
                    nc.vector.reciprocal(rst_all[:, g * 4 : g * 4 + 4], sd4)

        # ---------------- projections ----------------
        with (
            tc.tile_pool(name="pw", bufs=2) as pw,
            tc.tile_pool(name="ppj", bufs=4, space="PSUM") as ppj,
            tc.tile_pool(name="pnrm", bufs=4) as pnrm,
        ):
            def load_w_half(W, half):
                wt = pw.tile([128, NCD, 512], bf16, tag="W")
                nc.gpsimd.dma_start(
                    out=wt, in_=W[:, :, half * 512 : (half + 1) * 512]
                )
                return wt

            def proj_norm_tiles(W, half, n_tiles, dst, with_c8):
                """q/k projection + per-tile l2norm scale -> dst DRAM (bf16)."""
                wt = load_w_half(W, half)
                for tt in range(n_tiles):
                    r0 = tt * 128
                    kp = ppj.tile([128, 512], f32, tag="pj")
                    for c in range(NCD):
                        nc.tensor.matmul(
                            kp, lhsT=xnT[:, c, r0 : r0 + 128], rhs=wt[:, c, :],
                            start=(c == 0), stop=(c == NCD - 1),
                        )
                    sq = pnrm.tile([128, 512], f32, tag="sq")
                    nc.scalar.activation(out=sq, in_=kp, func=AF.Square)
                    nr = pnrm.tile([128, 8], f32, tag="nr")
                    nc.vector.tensor_reduce(
                        out=nr, in_=sq.rearrange("p (g d) -> p g d", g=8),
                        axis=AX.X, op=OP.add,
                    )
                    nc.scalar.activation(out=nr, in_=nr, func=AF.Sqrt, bias=eps30)
                    rs = pnrm.tile([128, 8], f32, tag="rs")
                    nc.vector.reciprocal(rs, nr)
                    nc.vector.tensor_scalar_min(out=rs, in0=rs, scalar1=1e12)
                    if with_c8:
                        rex = pnrm.tile([128, 8, DH], f32, tag="rex")
                        nc.vector.tensor_tensor(
                            out=rex,
                            in0=rs.broadcast_to([128, 8, DH]),
                            in1=c8, op=OP.mult,
                        )
                        knf = pnrm.tile([128, 512], bf16, tag="knf")
                        nc.vector.tensor_tensor(
                            out=knf.rearrange("p (g d) -> p g d", g=8),
                            in0=kp.rearrange("p (g d) -> p g d", g=8),
                            in1=rex, op=OP.mult,
                        )
                    else:
                        knf = pnrm.tile([128, 512], bf16, tag="knf")
                        nc.vector.tensor_tensor(
                            out=knf.rearrange("p (g d) -> p g d", g=8),
                            in0=kp.rearrange("p (g d) -> p g d", g=8),
                            in1=rs.broadcast_to([128, 8, DH]),
                            op=OP.mult,
                        )
                    nc.gpsimd.dma_start(
                        out=dst[r0 : r0 + 128, half * 512 : (half + 1) * 512],
                        in_=knf,
                    )

            def proj_v_tiles(half):
                wt = load_w_half(Wv, half)
                for tt in range(NT):
                    r0 = tt * 128
                    vp = ppj.tile([128, 512], f32, tag="pj")
                    for c in range(NCD):
                        nc.tensor.matmul(
                            vp, lhsT=xnT[:, c, r0 : r0 + 128], rhs=wt[:, c, :],
                            start=(c == 0), stop=(c == NCD - 1),
                        )
                    # v * rstd (per-token) on the scalar engine
                    nc.scalar.activation(
                        out=vsb[:, tt, half * 8 : (half + 1) * 8, 0:DH],
                        in_=vp.rearrange("p (g d) -> p g d", g=8),
                        func=AF.Copy, scale=rst_all[:, tt : tt + 1],
                    )

            for half in range(2):
                proj_norm_tiles(Wk, half, NT, kn_d, with_c8=False)
                proj_norm_tiles(Wq, half, NTQ, qn_d, with_c8=True)
                proj_v_tiles(half)
                # transposes for this half's head pairs
                for p in range(half * 4, half * 4 + 4):
                    nc.sync.dma_start(
                        out=kT[:, p, :], in_=kn_d[:, p * 128 : (p + 1) * 128],
                        transpose=True,
                    )
                    nc.sync.dma_start(
                        out=qT[:, p, :], in_=qn_d[:, p * 128 : (p + 1) * 128],
                        transpose=True,
                    )

        # ---------------- attention + interleaved output projection --------
        QB = NQ // 512  # 2 query blocks of 512
        with (
            tc.tile_pool(name="pstt", bufs=2, space="PSUM") as pstt,
            tc.tile_pool(name="pot", bufs=3, space="PSUM") as pot,
            tc.tile_pool(name="ppj2", bufs=1, space="PSUM") as ppj2,
            tc.tile_pool(name="ppt", bufs=3) as ppt,
            tc.tile_pool(name="pptn", bufs=2) as pptn,
            tc.tile_pool(name="prec", bufs=4) as prec,
            tc.tile_pool(name="pbsc", bufs=4) as pbsc,
            tc.tile_pool(name="pob", bufs=2) as pob,
        ):
            def outproj_block(i):
                """Output projection for the i-th [128-token x 512-col] block
                of qb0 (i in 0..7): token tile i//2, column half i%2."""
                tt, half = divmod(i, 2)
                r0 = tt * 128
                op_ = ppj2.tile([128, 512], f32)
                for c in range(NCD):
                    nc.tensor.matmul(
                        op_, lhsT=AT[:, c, r0 : r0 + 128],
                        rhs=wo_sb[:, c, half * 512 : (half + 1) * 512],
                        start=(c == 0), stop=(c == NCD - 1),
                    )
                ob = pob.tile([128, 512], f32)
                nc.vector.tensor_copy(out=ob, in_=op_)
                nc.sync.dma_start(
                    out=out[r0 : r0 + 128, half * 512 : (half + 1) * 512], in_=ob
                )

            for qb in range(QB):
                q0 = qb * 512
                for hp in range(HP):
                    hA, hB = 2 * hp, 2 * hp + 1
                    # null scores for both heads: [2, 512] (in an st slot)
                    st_n = pstt.tile([128, 2, 512], f32, tag="st")
                    null_ps = st_n[0:2, 0, :]
                    nc.tensor.matmul(
                        null_ps, lhsT=nkn_bd[:, hA : hA + 2],
                        rhs=qT[:, hp, q0 : q0 + 512], start=True, stop=True,
                    )
                    pTn = pptn.tile([2, 512], bf16)
                    nc.scalar.activation(out=pTn, in_=null_ps, func=AF.Exp, scale=SCALE)

                    otA = pot.tile([DH + 1, 512], f32, tag="ot")
                    otB = pot.tile([DH + 1, 512], f32, tag="ot")

                    for c in range(16):
                        st = pstt.tile([128, 2, 512], f32, tag="st")
                        for si, rh in ((0, 0), (1, 1)):
                            nc.tensor.matmul(
                                st[:, si, :],
                                lhsT=kT[rh * DH : (rh + 1) * DH, hp, c * 128 : (c + 1) * 128],
                                rhs=qT[rh * DH : (rh + 1) * DH, hp, q0 : q0 + 512],
                                start=True, stop=True,
                                tile_position=(rh * DH, 0),
                            )
                        pt = ppt.tile([128, 2, 512], bf16)
                        nc.scalar.activation(out=pt, in_=st, func=AF.Exp, scale=SCALE)
                        for ot, h in ((otA, hA), (otB, hB)):
                            nc.tensor.matmul(
                                ot, lhsT=vsb[:, c, h, :], rhs=pt[:, h % 2, :],
                                start=(c == 0), stop=False,
                            )
                    # null PV (finishes accumulation)
                    nc.tensor.matmul(
                        otA, lhsT=nv_bd2[:, hA, :], rhs=pTn, start=False, stop=True
                    )
                    nc.tensor.matmul(
                        otB, lhsT=nv_bd2[:, hB, :], rhs=pTn, start=False, stop=True
                    )
                    # reciprocal of denominators (row DH), broadcast on gpsimd,
                    # apply, and write A^T
                    denA = prec.tile([1, 512], f32, tag="denA")
                    denB = prec.tile([1, 512], f32, tag="denB")
                    nc.vector.tensor_copy(out=denA, in_=otA[DH : DH + 1, :])
                    nc.vector.tensor_copy(out=denB, in_=otB[DH : DH + 1, :])
                    rcpA = prec.tile([1, 512], f32, tag="rcpA")
                    rcpB = prec.tile([1, 512], f32, tag="rcpB")
                    nc.vector.reciprocal_approx_fast(out=rcpA, in_=denA)
                    nc.vector.reciprocal_approx_fast(out=rcpB, in_=denB)
                    for rcp1, (h, ot) in ((rcpA, (hA, otA)), (rcpB, (hB, otB))):
                        rcs = pbsc.tile([DH, 512], f32, tag="bcs")
                        nc.gpsimd.partition_broadcast(rcs, rcp1)
                        po = (h % 2) * DH
                        nc.vector.tensor_tensor(
                            out=AT[po : po + DH, h // 2, q0 : q0 + 512],
                            in0=ot[0:DH, :], in1=rcs, op=OP.mult,
                        )
                    # interleave qb0's output projection into qb1's attention
                    if qb == 1:
                        outproj_block(hp)
            # tail: qb1's output projection
            for i in range(8):
                tt, half = divmod(i, 2)
                r0 = 512 + tt * 128
                op_ = ppj2.tile([128, 512], f32)
                for c in range(NCD):
                    nc.tensor.matmul(
                        op_, lhsT=AT[:, c, r0 : r0 + 128],
                        rhs=wo_sb[:, c, half * 512 : (half + 1) * 512],
                        start=(c == 0), stop=(c == NCD - 1),
                    )
                ob = pob.tile([128, 512], f32)
                nc.vector.tensor_copy(out=ob, in_=op_)
                nc.sync.dma_start(
                    out=out[r0 : r0 + 128, half * 512 : (half + 1) * 512], in_=ob
                )

    nc.compile()
    return nc


def _get_program(beta_zero: bool = True):
    key = "nc_v2"
    if key not in _CACHE:
        _CACHE[key] = _build_program()
    return _CACHE[key]


def _prep_weights(Wq, Wkv, Wo, gamma, beta):
    """Host-side: fold gamma into the projection weights, pre-tile to
    [128, NCD, cols] (chunk c holds dim rows c*128:(c+1)*128), cast bf16."""
    import ml_dtypes

    NCD = DIM // 128
    Wk = Wkv[:, :INNER]
    Wv = Wkv[:, INNER:]
    g = gamma.astype(np.float64)[:, None]

    def tile_w(W, fold_gamma=True):
        Wf = W.astype(np.float64) * g if fold_gamma else W.astype(np.float64)
        t = Wf.reshape(NCD, 128, Wf.shape[1]).transpose(1, 0, 2)
        return np.ascontiguousarray(t.astype(ml_dtypes.bfloat16))

    return (
        tile_w(Wq), tile_w(Wk), tile_w(Wv), tile_w(Wo, fold_gamma=False),
    )


def kernel(**inputs) -> np.ndarray:
    from concourse.bass_utils import run_bass_kernel_spmd

    x = np.asarray(inputs["x"], dtype=np.float32)
    gamma = np.asarray(inputs["gamma"], dtype=np.float32)
    beta = np.asarray(inputs["beta"], dtype=np.float32)
    null_kv = np.asarray(inputs["null_kv"], dtype=np.float32)
    Wq = np.asarray(inputs["Wq"], dtype=np.float32)
    Wkv = np.asarray(inputs["Wkv"], dtype=np.float32)
    q_scale = np.asarray(inputs["q_scale"], dtype=np.float32)
    k_scale = np.asarray(inputs["k_scale"], dtype=np.float32)
    Wo = np.asarray(inputs["Wo"], dtype=np.float32)

    if np.any(beta):
        # General-path fallback: beta shifts xn, which the fast path folds
        # away. Absorb beta into x directly: xn = ((x-mu)/std)*gamma + beta
        # is NOT linear in x, so instead fall back to adding beta/gamma
        # pre-projection is invalid; handle by explicit correction below.
        raise NotImplementedError("beta != 0 not supported by this kernel")

    nc = _get_program()
    Wq_t, Wk_t, Wv_t, Wo_t = _prep_weights(Wq, Wkv, Wo, gamma, beta)
    nk = np.ascontiguousarray(null_kv[0, :, 0, :])
    nv = np.ascontiguousarray(null_kv[1, :, 0, :])

    in_maps = []
    for b in range(B):
        for hi in range(2):
            xb = x[b]
            if hi == 1:
                xb = np.concatenate([xb[NQ:], xb[:NQ]], axis=0)
            in_maps.append(
                {
                    "x": np.ascontiguousarray(xb),
                    "Wq": Wq_t,
                    "Wk": Wk_t,
                    "Wv": Wv_t,
                    "Wo": Wo_t,
                    "nk": nk,
                    "nv": nv,
                    "qs": q_scale,
                    "ks": k_scale,
                }
            )

    res = run_bass_kernel_spmd(nc, in_maps, list(range(8)))

    full = np.empty((B, N, DIM), dtype=np.float32)
    for c in range(8):
        b, hi = divmod(c, 2)
        full[b, hi * NQ : (hi + 1) * NQ] = res.results[c]["out"]
    return full


# revision 17
# speedup vs baseline: 1.5365x; 1.0615x over previous
"""Trainium2 Bass kernel for nn_Attention (LayerNorm + L2-normalized-QK attention
with null-kv slot + output projection), SPMD across 8 NeuronCores.

Sharding: core c = (batch b = c//2, query-half hi = c%2). Each core computes the
full kv (2048 tokens) of its batch and attention outputs for its 1024-query
half. Softmax over kv is permutation invariant, so for hi=1 we feed x with the
two sequence halves swapped — every core then runs the identical SPMD program
with its queries in rows 0:1024. The final output is a pure concatenation of
the per-core results (no collectives, no host arithmetic on outputs).

v2 pipeline layout (vs v1): LayerNorm is pipelined per 512-token group into
the projections so the PE never idles at the head; gamma is folded into the
projection weights on the host and weights are shipped pre-tiled in bf16 (no
on-device weight casts); the l2norm rsqrt runs per tile (Ln+Exp live in one
ACT table set); PSUM->SBUF projection copies are fused into the normalization
multiplies; v-scaling moves to the scalar engine; the softmax denominator
broadcast runs on the (otherwise idle) gpsimd engine instead of DRAM
round-trips; and the qb0 output projection is interleaved into qb1 attention.
"""

import numpy as np

B = 4
N = 2048
DIM = 1024
HEADS = 16
DH = 64
INNER = HEADS * DH
NQ = 1024  # queries per core
SCALE = 8.0
LN_EPS = 1e-5

_CACHE = {}


def _build_program():
    from contextlib import ExitStack

    import concourse.bacc as bacc
    import concourse.tile as tile
    from concourse import mybir

    f32 = mybir.dt.float32
    bf16 = mybir.dt.bfloat16
    i16 = mybir.dt.int16
    AF = mybir.ActivationFunctionType
    OP = mybir.AluOpType
    AX = mybir.AxisListType

    NT = N // 128          # 16 token tiles
    NTQ = NQ // 128        # 8 query token tiles
    NCD = DIM // 128       # 8 dim chunks
    HP = HEADS // 2        # 8 head pairs
    NG = 4                 # 512-token LN groups

    nc = bacc.Bacc("TRN2", target_bir_lowering=False, debug=False)

    x = nc.declare_dram_parameter("x", [N, DIM], f32, isOutput=False)
    # weights arrive pre-tiled [128, NCD, INNER] bf16, gamma pre-folded
    Wq = nc.declare_dram_parameter("Wq", [128, NCD, INNER], bf16, isOutput=False)
    Wk = nc.declare_dram_parameter("Wk", [128, NCD, INNER], bf16, isOutput=False)
    Wv = nc.declare_dram_parameter("Wv", [128, NCD, INNER], bf16, isOutput=False)
    Wo = nc.declare_dram_parameter("Wo", [128, NCD, DIM], bf16, isOutput=False)
    nk = nc.declare_dram_parameter("nk", [HEADS, DH], f32, isOutput=False)
    nv = nc.declare_dram_parameter("nv", [HEADS, DH], f32, isOutput=False)
    qs = nc.declare_dram_parameter("qs", [DH], f32, isOutput=False)
    ks = nc.declare_dram_parameter("ks", [DH], f32, isOutput=False)
    out = nc.declare_dram_parameter("out", [NQ, DIM], f32, isOutput=True)

    # internal DRAM for transpose round-trips
    xn_d = nc.dram_tensor("xn_d", [N, DIM], bf16)
    kn_d = nc.dram_tensor("kn_d", [N, INNER], bf16)
    qn_d = nc.dram_tensor("qn_d", [NQ, INNER], bf16)
    nkn_d = nc.dram_tensor("nkn_d", [HEADS, DH], bf16)
    nvb_d = nc.dram_tensor("nvb_d", [HEADS, DH + 1], bf16)

    with tile.TileContext(nc) as tc, ExitStack() as ctx:
        singles = ctx.enter_context(tc.tile_pool(name="singles", bufs=1))
        big = ctx.enter_context(tc.tile_pool(name="big", bufs=1))

        # ---------------- persistent SBUF tensors ----------------
        xnT = big.tile([128, NCD, N], bf16, tag="xnT")       # (x-mu)^T [dim, tok]
        kT = big.tile([128, HP, N], bf16, tag="kT")          # k^T   [2*64, kv] per pair
        qT = big.tile([128, HP, NQ], bf16, tag="qT")         # q^T
        vsb = big.tile([128, NT, HEADS, DH + 1], bf16, tag="v")   # V'=[V|1]
        AT = big.tile([128, NCD, NQ], bf16, tag="AT")        # A^T (attn out)
        wo_sb = big.tile([128, NCD, DIM], bf16, tag="wo")    # Wo resident

        # ---------------- constants ----------------
        eps_t = singles.tile([128, 1], f32)
        nc.vector.memset(eps_t, LN_EPS)
        eps30 = singles.tile([128, 1], f32)
        nc.vector.memset(eps30, 1e-30)

        qs_b = singles.tile([128, DH], f32)
        nc.gpsimd.dma_start(out=qs_b, in_=qs.ap().partition_broadcast(128))
        ks_b = singles.tile([128, DH], f32)
        nc.gpsimd.dma_start(out=ks_b, in_=ks.ap().partition_broadcast(128))
        c64 = singles.tile([128, DH], f32)
        nc.vector.tensor_tensor(out=c64, in0=qs_b, in1=ks_b, op=OP.mult)
        c8 = singles.tile([128, 8, DH], f32)   # qs*ks tiled for 8 heads (one col half)
        for g in range(8):
            nc.vector.tensor_copy(out=c8[:, g, :], in_=c64)

        nc.vector.memset(vsb[:, :, :, DH : DH + 1], 1.0)  # ones column of V'

        # Wo resident load (2 DMAs on the gpsimd queue, early)
        for half in range(2):
            nc.gpsimd.dma_start(
                out=wo_sb[:, :, half * 512 : (half + 1) * 512],
                in_=Wo[:, :, half * 512 : (half + 1) * 512],
            )

        # null-kv prep: nkn = l2norm(nk)*qs*ks (bf16), then transpose via DRAM
        nk_t = singles.tile([HEADS, DH], f32)
        nc.sync.dma_start(out=nk_t, in_=nk[:, :])
        nksq = singles.tile([HEADS, DH], f32)
        nc.vector.tensor_tensor(out=nksq, in0=nk_t, in1=nk_t, op=OP.mult)
        nks = singles.tile([HEADS, 1], f32)
        nc.vector.tensor_reduce(out=nks, in_=nksq, axis=AX.X, op=OP.add)
        nkr = singles.tile([HEADS, 1], f32)
        nc.scalar.activation(out=nkr, in_=nks, func=AF.Sqrt, bias=eps30[0:HEADS, :])
        nc.vector.reciprocal(nks, nkr)
        nc.vector.tensor_scalar_min(out=nks, in0=nks, scalar1=1e12)
        nkn = singles.tile([HEADS, DH], f32)
        nc.vector.tensor_scalar_mul(out=nkn, in0=nk_t, scalar1=nks)
        nknb = singles.tile([HEADS, DH], bf16)
        nc.vector.tensor_tensor(out=nknb, in0=nkn, in1=c64[0:HEADS, :], op=OP.mult)
        nc.sync.dma_start(out=nkn_d[:, :], in_=nknb)
        nknT = singles.tile([DH, HEADS], bf16)
        nc.sync.dma_start(out=nknT, in_=nkn_d.ap().rearrange("h d -> d h"))
        # block-diagonal [128, 16]: col 2p rows 0:64 = head 2p, col 2p+1 rows 64:128 = head 2p+1
        nkn_bd = singles.tile([128, HEADS], bf16)
        nc.vector.memset(nkn_bd, 0.0)
        nc.sync.dma_start(out=nkn_bd[0:DH, 0:HEADS:2], in_=nknT[:, 0:HEADS:2])
        nc.sync.dma_start(out=nkn_bd[DH:128, 1:HEADS:2], in_=nknT[:, 1:HEADS:2])

        # null-v: nv_bd2 [2, HEADS, DH+1]; row parity selects head parity
        nv_t = singles.tile([HEADS, DH], f32)
        nc.sync.dma_start(out=nv_t, in_=nv[:, :])
        nvb = singles.tile([HEADS, DH + 1], bf16)
        nc.vector.tensor_copy(out=nvb[:, 0:DH], in_=nv_t)
        nc.vector.memset(nvb[:, DH : DH + 1], 1.0)
        nv_bd2 = singles.tile([2, HEADS, DH + 1], bf16)
        nc.vector.memset(nv_bd2, 0.0)
        nc.sync.dma_start(out=nvb_d[:, :], in_=nvb)
        nc.sync.dma_start(
            out=nv_bd2[0:1, 0:HEADS:2, :],
            in_=nvb_d.ap()[0:HEADS:2, :].partition_broadcast(1),
        )
        nc.sync.dma_start(
            out=nv_bd2[1:2, 1:HEADS:2, :],
            in_=nvb_d.ap()[1:HEADS:2, :].partition_broadcast(1),
        )

        # ---------------- LayerNorm, pipelined per 512-token group ----------
        # beta==0 fast path: xnu = x - mu. gamma is folded into the weights;
        # the per-token rstd commutes through the projections (q/k l2norms
        # cancel it; the v-proj re-applies it as a per-partition scale).
        mv_all = singles.tile([128, NT, 2], f32)
        rst_all = singles.tile([128, NT], f32)
        with (
            tc.tile_pool(name="px", bufs=5) as px,
            tc.tile_pool(name="pst", bufs=4) as pst,
            tc.tile_pool(name="pxn", bufs=6) as pxn,
        ):
            for tt in range(NT):
                r0 = tt * 128
                xt = px.tile([128, DIM], f32)
                nc.gpsimd.dma_start(out=xt, in_=x[r0 : r0 + 128, :])
                stats = pst.tile([128, 2, 6], f32, tag="stats")
                nc.vector.bn_stats(out=stats[:, 0, :], in_=xt[:, 0:512])
                nc.vector.bn_stats(out=stats[:, 1, :], in_=xt[:, 512:1024])
                nc.vector.bn_aggr(out=mv_all[:, tt, :], in_=stats)
                xnt = pxn.tile([128, DIM], bf16)
                nc.vector.tensor_scalar_sub(
                    out=xnt, in0=xt, scalar1=mv_all[:, tt, 0:1]
                )
                nc.sync.dma_start(out=xn_d[r0 : r0 + 128, :], in_=xnt)
                if tt % 4 == 3:
                    g = tt // 4
                    g0 = (tt - 3) * 128
                    # one batched xbar transpose for the whole 512-token group
                    nc.scalar.dma_start(
                        out=xnT[:, :, g0 : g0 + 512],
                        in_=xn_d[g0 : g0 + 512, :],
                        transpose=True,
                    )
                    # rstd for this group's 4 tiles (only v-proj consumes it)
                    sd4 = pst.tile([128, 4], f32, tag="sd4")
                    nc.scalar.activation(
                        out=sd4,
                        in_=mv_all[:, g * 4 : g * 4 + 4, 1],
                        func=AF.Sqrt, bias=eps_t,
                    )
                    nc.vector.reciprocal(rst_all[:, g * 4 : g * 4 + 4], sd4)

        # ---------------- projections ----------------
        with (
            tc.tile_pool(name="pw", bufs=2) as pw,
            tc.tile_pool(name="ppj", bufs=4, space="PSUM") as ppj,
            tc.tile_pool(name="pnrm", bufs=4) as pnrm,
        ):
            def load_w_half(W, half):
                wt = pw.tile([128, NCD, 512], bf16, tag="W")
                nc.gpsimd.dma_start(
                    out=wt, in_=W[:, :, half * 512 : (half + 1) * 512]
                )
                return wt

            def proj_norm_tiles(W, half, n_tiles, dst, with_c8):
                """q/k projection + per-tile l2norm scale -> dst DRAM (bf16)."""
                wt = load_w_half(W, half)
                for tt in range(n_tiles):
                    r0 = tt * 128
                    kp = ppj.tile([128, 512], f32, tag="pj")
                    for c in range(NCD):
                        nc.tensor.matmul(
                            kp, lhsT=xnT[:, c, r0 : r0 + 128], rhs=wt[:, c, :],
                            start=(c == 0), stop=(c == NCD - 1),
                        )
                    sq = pnrm.tile([128, 512], f32, tag="sq")
                    nc.scalar.activation(out=sq, in_=kp, func=AF.Square)
                    nr = pnrm.tile([128, 8], f32, tag="nr")
                    nc.vector.tensor_reduce(
                        out=nr, in_=sq.rearrange("p (g d) -> p g d", g=8),
                        axis=AX.X, op=OP.add,
                    )
                    nc.scalar.activation(out=nr, in_=nr, func=AF.Sqrt, bias=eps30)
                    rs = pnrm.tile([128, 8], f32, tag="rs")
                    nc.vector.reciprocal(rs, nr)
                    nc.vector.tensor_scalar_min(out=rs, in0=rs, scalar1=1e12)
                    if with_c8:
                        rex = pnrm.tile([128, 8, DH], f32, tag="rex")
                        nc.vector.tensor_tensor(
                            out=rex,
                            in0=rs.broadcast_to([128, 8, DH]),
                            in1=c8, op=OP.mult,
                        )
                        knf = pnrm.tile([128, 512], bf16, tag="knf")
                        nc.vector.tensor_tensor(
                            out=knf.rearrange("p (g d) -> p g d", g=8),
                            in0=kp.rearrange("p (g d) -> p g d", g=8),
                            in1=rex, op=OP.mult,
                        )
                    else:
                        knf = pnrm.tile([128, 512], bf16, tag="knf")
                        nc.vector.tensor_tensor(
                            out=knf.rearrange("p (g d) -> p g d", g=8),
                            in0=kp.rearrange("p (g d) -> p g d", g=8),
                            in1=rs.broadcast_to([128, 8, DH]),
                            op=OP.mult,
                        )
                    nc.gpsimd.dma_start(
                        out=dst[r0 : r0 + 128, half * 512 : (half + 1) * 512],
                        in_=knf,
                    )

            def proj_v_tiles(half):
                wt = load_w_half(Wv, half)
                for tt in range(NT):
                    r0 = tt * 128
                    vp = ppj.tile([128, 512], f32, tag="pj")
                    for c in range(NCD):
                        nc.tensor.matmul(
                            vp, lhsT=xnT[:, c, r0 : r0 + 128], rhs=wt[:, c, :],
                            start=(c == 0), stop=(c == NCD - 1),
                        )
                    # v * rstd (per-token) on the scalar engine
                    nc.scalar.activation(
                        out=vsb[:, tt, half * 8 : (half + 1) * 8, 0:DH],
                        in_=vp.rearrange("p (g d) -> p g d", g=8),
                        func=AF.Copy, scale=rst_all[:, tt : tt + 1],
                    )

            for half in range(2):
                proj_norm_tiles(Wk, half, NT, kn_d, with_c8=False)
                proj_norm_tiles(Wq, half, NTQ, qn_d, with_c8=True)
                proj_v_tiles(half)
                # transposes for this half's head pairs
                for p in range(half * 4, half * 4 + 4):
                    nc.sync.dma_start(
                        out=kT[:, p, :], in_=kn_d[:, p * 128 : (p + 1) * 128],
                        transpose=True,
                    )
                    nc.sync.dma_start(
                        out=qT[:, p, :], in_=qn_d[:, p * 128 : (p + 1) * 128],
                        transpose=True,
                    )

        # ---------------- attention + interleaved output projection --------
        QB = NQ // 512  # 2 query blocks of 512
        with (
            tc.tile_pool(name="pstt", bufs=2, space="PSUM") as pstt,
            tc.tile_pool(name="pot", bufs=3, space="PSUM") as pot,
            tc.tile_pool(name="ppj2", bufs=1, space="PSUM") as ppj2,
            tc.tile_pool(name="ppt", bufs=3) as ppt,
            tc.tile_pool(name="pptn", bufs=2) as pptn,
            tc.tile_pool(name="prec", bufs=2) as prec,
            tc.tile_pool(name="pbsc", bufs=2) as pbsc,
            tc.tile_pool(name="pob", bufs=2) as pob,
        ):
            def outproj_block(i):
                """Output projection for the i-th [128-token x 512-col] block
                of qb0 (i in 0..7): token tile i//2, column half i%2."""
                tt, half = divmod(i, 2)
                r0 = tt * 128
                op_ = ppj2.tile([128, 512], f32)
                for c in range(NCD):
                    nc.tensor.matmul(
                        op_, lhsT=AT[:, c, r0 : r0 + 128],
                        rhs=wo_sb[:, c, half * 512 : (half + 1) * 512],
                        start=(c == 0), stop=(c == NCD - 1),
                    )
                ob = pob.tile([128, 512], f32)
                nc.vector.tensor_copy(out=ob, in_=op_)
                nc.sync.dma_start(
                    out=out[r0 : r0 + 128, half * 512 : (half + 1) * 512], in_=ob
                )

            for qb in range(QB):
                q0 = qb * 512
                for hp in range(HP):
                    hA, hB = 2 * hp, 2 * hp + 1
                    # null scores for both heads: [2, 512] (in an st slot)
                    st_n = pstt.tile([128, 2, 512], f32, tag="st")
                    null_ps = st_n[0:2, 0, :]
                    nc.tensor.matmul(
                        null_ps, lhsT=nkn_bd[:, hA : hA + 2],
                        rhs=qT[:, hp, q0 : q0 + 512], start=True, stop=True,
                    )
                    pTn = pptn.tile([2, 512], bf16)
                    nc.scalar.activation(out=pTn, in_=null_ps, func=AF.Exp, scale=SCALE)

                    otA = pot.tile([DH + 1, 512], f32, tag="ot")
                    otB = pot.tile([DH + 1, 512], f32, tag="ot")

                    for c in range(16):
                        st = pstt.tile([128, 2, 512], f32, tag="st")
                        for si, rh in ((0, 0), (1, 1)):
                            nc.tensor.matmul(
                                st[:, si, :],
                                lhsT=kT[rh * DH : (rh + 1) * DH, hp, c * 128 : (c + 1) * 128],
                                rhs=qT[rh * DH : (rh + 1) * DH, hp, q0 : q0 + 512],
                                start=True, stop=True,
                                tile_position=(rh * DH, 0),
                            )
                        if c % 4 == 3:
                            # Schraudolph exp on the vector engine: bf16 bit
                            # pattern of exp(8*x) is ~ x*8*128/ln2 + 16248
                            # (int16), ~1.6% weight noise, softmax-tolerant.
                            pt16 = ppt.tile([128, 2, 512], i16, tag="pt16")
                            nc.vector.tensor_scalar(
                                out=pt16, in0=st,
                                scalar1=SCALE * 184.6649652337873,
                                scalar2=16248.0,
                                op0=OP.mult, op1=OP.add,
                            )
                            pt = pt16.bitcast(bf16)
                        else:
                            pt = ppt.tile([128, 2, 512], bf16, tag="pt")
                            nc.scalar.activation(out=pt, in_=st, func=AF.Exp, scale=SCALE)
                        for ot, h in ((otA, hA), (otB, hB)):
                            nc.tensor.matmul(
                                ot, lhsT=vsb[:, c, h, :], rhs=pt[:, h % 2, :],
                                start=(c == 0), stop=False,
                            )
                    # null PV (finishes accumulation)
                    nc.tensor.matmul(
                        otA, lhsT=nv_bd2[:, hA, :], rhs=pTn, start=False, stop=True
                    )
                    nc.tensor.matmul(
                        otB, lhsT=nv_bd2[:, hB, :], rhs=pTn, start=False, stop=True
                    )
                    # reciprocal of denominators (row DH), broadcast on gpsimd,
                    # apply, and write A^T
                    denA = prec.tile([1, 512], f32, tag="denA")
                    denB = prec.tile([1, 512], f32, tag="denB")
                    nc.vector.tensor_copy(out=denA, in_=otA[DH : DH + 1, :])
                    nc.vector.tensor_copy(out=denB, in_=otB[DH : DH + 1, :])
                    rcpA = prec.tile([1, 512], f32, tag="rcpA")
                    rcpB = prec.tile([1, 512], f32, tag="rcpB")
                    nc.vector.reciprocal_approx_fast(out=rcpA, in_=denA)
                    nc.vector.reciprocal_approx_fast(out=rcpB, in_=denB)
                    for rcp1, (h, ot) in ((rcpA, (hA, otA)), (rcpB, (hB, otB))):
                        rcs = pbsc.tile([DH, 512], f32, tag="bcs")
                        nc.gpsimd.partition_broadcast(rcs, rcp1)
                        po = (h % 2) * DH
                        nc.vector.tensor_tensor(
                            out=AT[po : po + DH, h // 2, q0 : q0 + 512],
                            in0=ot[0:DH, :], in1=rcs, op=OP.mult,
                        )
                    # interleave qb0's output projection into qb1's attention
                    if qb == 1:
                        outproj_block(hp)
            # tail: qb1's output projection
            for i in range(8):
                tt, half = divmod(i, 2)
                r0 = 512 + tt * 128
                op_ = ppj2.tile([128, 512], f32)
                for c in range(NCD):
                    nc.tensor.matmul(
                        op_, lhsT=AT[:, c, r0 : r0 + 128],
                        rhs=wo_sb[:, c, half * 512 : (half + 1) * 512],
                        start=(c == 0), stop=(c == NCD - 1),
                    )
                ob = pob.tile([128, 512], f32)
                nc.vector.tensor_copy(out=ob, in_=op_)
                nc.sync.dma_start(
                    out=out[r0 : r0 + 128, half * 512 : (half + 1) * 512], in_=ob
                )

    nc.compile()
    return nc


def _get_program(beta_zero: bool = True):
    key = "nc_v2"
    if key not in _CACHE:
        _CACHE[key] = _build_program()
    return _CACHE[key]


def _prep_weights(Wq, Wkv, Wo, gamma, beta):
    """Host-side: fold gamma into the projection weights, pre-tile to
    [128, NCD, cols] (chunk c holds dim rows c*128:(c+1)*128), cast bf16."""
    import ml_dtypes

    NCD = DIM // 128
    Wk = Wkv[:, :INNER]
    Wv = Wkv[:, INNER:]
    g = gamma.astype(np.float64)[:, None]

    def tile_w(W, fold_gamma=True):
        Wf = W.astype(np.float64) * g if fold_gamma else W.astype(np.float64)
        t = Wf.reshape(NCD, 128, Wf.shape[1]).transpose(1, 0, 2)
        return np.ascontiguousarray(t.astype(ml_dtypes.bfloat16))

    return (
        tile_w(Wq), tile_w(Wk), tile_w(Wv), tile_w(Wo, fold_gamma=False),
    )


def kernel(**inputs) -> np.ndarray:
    from concourse.bass_utils import run_bass_kernel_spmd

    x = np.asarray(inputs["x"], dtype=np.float32)
    gamma = np.asarray(inputs["gamma"], dtype=np.float32)
    beta = np.asarray(inputs["beta"], dtype=np.float32)
    null_kv = np.asarray(inputs["null_kv"], dtype=np.float32)
    Wq = np.asarray(inputs["Wq"], dtype=np.float32)
    Wkv = np.asarray(inputs["Wkv"], dtype=np.float32)
    q_scale = np.asarray(inputs["q_scale"], dtype=np.float32)
    k_scale = np.asarray(inputs["k_scale"], dtype=np.float32)
    Wo = np.asarray(inputs["Wo"], dtype=np.float32)

    if np.any(beta):
        # General-path fallback: beta shifts xn, which the fast path folds
        # away. Absorb beta into x directly: xn = ((x-mu)/std)*gamma + beta
        # is NOT linear in x, so instead fall back to adding beta/gamma
        # pre-projection is invalid; handle by explicit correction below.
        raise NotImplementedError("beta != 0 not supported by this kernel")

    nc = _get_program()
    Wq_t, Wk_t, Wv_t, Wo_t = _prep_weights(Wq, Wkv, Wo, gamma, beta)
    nk = np.ascontiguousarray(null_kv[0, :, 0, :])
    nv = np.ascontiguousarray(null_kv[1, :, 0, :])

    in_maps = []
    for b in range(B):
        for hi in range(2):
            xb = x[b]
            if hi == 1:
                xb = np.concatenate([xb[NQ:], xb[:NQ]], axis=0)
            in_maps.append(
                {
                    "x": np.ascontiguousarray(xb),
                    "Wq": Wq_t,
                    "Wk": Wk_t,
                    "Wv": Wv_t,
                    "Wo": Wo_t,
                    "nk": nk,
                    "nv": nv,
                    "qs": q_scale,
                    "ks": k_scale,
                }
            )

    res = run_bass_kernel_spmd(nc, in_maps, list(range(8)))

    full = np.empty((B, N, DIM), dtype=np.float32)
    for c in range(8):
        b, hi = divmod(c, 2)
        full[b, hi * NQ : (hi + 1) * NQ] = res.results[c]["out"]
    return full
